# revision 11
# baseline (speedup 1.0000x reference)
"""Trainium2 Bass kernel for the Mamba U-Net model (nn_Model_20770461843918).

Batch-data-parallel SPMD over 8 NeuronCores (4 batch elements; cores c and
c+4 duplicate work, outputs read from cores 0-3).  Per core the whole
7-block Mamba U-Net runs locally with partitions = inner channel d.

v2 layout (fp16 compute, fp32 PSUM / scan state):
  PE  : all matmuls in fp16 (1 cyc/col): in/x/dt/out projections, depthwise
        conv via diagonal matmuls, down/up/gate convs, K=1 ones-matmul
        broadcast of per-timestep B/C rows (shared across both d-groups)
  ACT : silu/exp/ln activations, PSUM->SBUF copies (table-swap minimized:
        stage M split into M1 in-proj / M2 conv+xproj / M3 dt passes)
  Pool: PSUM->SBUF fp16 copies of the B/C broadcasts (frees DVE 2x mode)
  DVE : dtu, dBu = dtu*Brep (1 instr, 3D AP), 16 tensor_tensor_scan
        (fp16 operands, fp32 internal state), h*Crep (1 instr), tree-reduce
"""
import numpy as np

B, L0, C = 4, 1024, 128
DI, NST, R, KC = 256, 16, 8, 4
NV = NST + 3          # packed per-partition vec cols: A[16], D, convb, bdt
NCORES = 8
TS = 512              # scan-stage time chunk
MM = 512              # matmul-stage time chunk

_CACHE = {}


def _prep_weights(inp):
    import ml_dtypes
    f32, f16 = np.float32, ml_dtypes.bfloat16
    g = lambda k: np.asarray(inp[k], f32)
    m_Win, m_convw, m_convb = g("m_Win"), g("m_convw"), g("m_convb")
    m_Wx, m_Wdt, m_bdt = g("m_Wx"), g("m_Wdt"), g("m_bdt")
    m_Alog, m_D, m_Wout = g("m_Alog"), g("m_D"), g("m_Wout")
    dc_w, dc_b = g("dc_w"), g("dc_b")
    wg_W, wg_b, db_W, db_b = g("wg_W"), g("wg_b"), g("db_W"), g("db_b")
    up_w, up_b = g("up_w"), g("up_b")

    winT = np.ascontiguousarray(m_Win.transpose(0, 2, 1)).astype(f16)  # [7,C,512]
    cd = np.zeros((7, 2, KC, 128, 128), f32)
    idx = np.arange(128)
    for i in range(7):
        for gg in range(2):
            for k in range(KC):
                cd[i, gg, k, idx, idx] = m_convw[i, gg * 128:(gg + 1) * 128, k]
    convdiag = np.ascontiguousarray(
        cd.transpose(0, 1, 3, 2, 4)).reshape(7, 2, 128, KC * 128).astype(f16)
    wxT_raw = np.ascontiguousarray(m_Wx.transpose(0, 2, 1)).reshape(7, 2, 128, R + 2 * NST)
    wxT = np.zeros((7, 2, 128, 64), f32)
    wxT[..., :R] = wxT_raw[..., :R]          # dt rows -> psum partitions 0..7
    wxT[..., 32:64] = wxT_raw[..., R:]       # B/C rows -> psum partitions 32..63
    wdtT = np.ascontiguousarray(m_Wdt.transpose(0, 2, 1))                # [7, R, DI]
    wdtall = wdtT.transpose(1, 0, 2).reshape(R, 7 * DI).astype(f16)     # [8, 7*256]
    woutT = np.ascontiguousarray(m_Wout.transpose(0, 2, 1)).reshape(7, 2, 128, C)
    dcwT = np.ascontiguousarray(dc_w.transpose(0, 2, 3, 1)).reshape(3, 128, 3 * 128)
    upw = np.ascontiguousarray(up_w.transpose(0, 1, 3, 2)).reshape(3, 128, 2 * 128)
    wgT = np.ascontiguousarray(wg_W.transpose(0, 2, 1)).reshape(3, 2, 128, 128)
    dbT = np.ascontiguousarray(db_W.transpose(0, 2, 1)).reshape(3, 2, 128, 128)

    # fp16 matmul-weight panels, order must match _build
    panels = []
    for i in range(7):
        panels += [wxT[i, 0], wxT[i, 1], woutT[i, 0], woutT[i, 1]]
    for j in range(3):
        panels += [dcwT[j], upw[j], wgT[j, 0], wgT[j, 1], dbT[j, 0], dbT[j, 1]]
    wtpack = np.concatenate(panels, axis=1).astype(f16)

    # fp32 per-partition scalar columns (act scale/bias, stt scalars)
    A = -np.exp(m_Alog)                                                  # [7, DI, N]
    vec = np.zeros((7, 2, 128, NV), f32)
    for gg in range(2):
        sl = slice(gg * 128, (gg + 1) * 128)
        vec[:, gg, :, :NST] = A[:, sl, :]
        vec[:, gg, :, NST] = m_D[:, sl]
        vec[:, gg, :, NST + 1] = m_convb[:, sl]
        vec[:, gg, :, NST + 2] = m_bdt[:, sl]
    gv = np.zeros((3, 128, 4), f32)
    gv[:, :, 0], gv[:, :, 1], gv[:, :, 2], gv[:, :, 3] = dc_b, up_b, wg_b, db_b
    vecpack = np.concatenate(
        [vec.transpose(0, 1, 3, 2).reshape(7 * 2 * NV, 128).T,
         gv.transpose(0, 2, 1).reshape(12, 128).T], axis=1)

    return {"winT": np.ascontiguousarray(winT),
            "convdiag": np.ascontiguousarray(convdiag),
            "wdtall": np.ascontiguousarray(wdtall),
            "wtpack": np.ascontiguousarray(wtpack),
            "vecpack": np.ascontiguousarray(vecpack.astype(f32))}


def _build():
    import concourse.bacc as bacc
    import concourse.tile as tile
    import concourse.mybir as mybir

    F32 = mybir.dt.float32
    F16 = mybir.dt.bfloat16
    Alu = mybir.AluOpType
    Act = mybir.ActivationFunctionType

    nc = bacc.Bacc("TRN2", target_bir_lowering=False, debug=False,
                   num_devices=NCORES)

    xT_d = nc.declare_dram_parameter("xT", [C, L0], F16, isOutput=False)
    out_d = nc.declare_dram_parameter("out", [C, L0], F32, isOutput=True)
    BLKW, GATW = 384, 1152
    TOTW = 7 * BLKW + 3 * GATW
    NVEC = 7 * 2 * NV + 3 * 4
    dram = {}
    for name, shape, dt in [
        ("winT", [7, C, 2 * DI], F16), ("convdiag", [7, 2, 128, KC * 128], F16),
        ("wdtall", [R, 7 * DI], F16), ("wtpack", [128, TOTW], F16),
        ("vecpack", [128, NVEC], F32),
    ]:
        dram[name] = nc.declare_dram_parameter(name, shape, dt, isOutput=False)
    bc_dram2 = [nc.dram_tensor("bc_bounce0", [2 * NST, L0], F16),
                nc.dram_tensor("bc_bounce1", [2 * NST, L0], F16)]

    with tile.TileContext(nc) as tc:
        with tc.tile_pool(name="wt", bufs=1) as wt, \
             tc.tile_pool(name="lvl", bufs=1) as lvl, \
             tc.tile_pool(name="blk", bufs=1) as blk, \
             tc.tile_pool(name="cube", bufs=1) as cube, \
             tc.tile_pool(name="cw", bufs=2) as cw, \
             tc.tile_pool(name="ubuf", bufs=2) as ubuf, \
             tc.tile_pool(name="gw", bufs=2) as gw, \
             tc.tile_pool(name="cwc", bufs=2) as cwc, \
             tc.tile_pool(name="bczp", bufs=1) as bczp, \
             tc.tile_pool(name="mbp", bufs=1) as mbp, \
             tc.tile_pool(name="mmp", bufs=5, space="PSUM") as mmp, \
             tc.tile_pool(name="xdbp", bufs=2, space="PSUM") as xdbp:

            def load_blk(i):
                winTb = cw.tile([C, 2 * DI], F16, tag="winT", name=f"winTb{i}")
                nc.scalar.dma_start(winTb[:], dram["winT"][i])
                cdw = cwc.tile([128, 2 * KC * 128], F16, tag="convdiag",
                               name=f"cdw{i}")
                nc.scalar.dma_start(cdw[:, :KC * 128], dram["convdiag"][i, 0])
                nc.scalar.dma_start(cdw[:, KC * 128:], dram["convdiag"][i, 1])
                return cdw, winTb

            preload = {0: load_blk(0)}

            wtall = wt.tile([128, TOTW], F16, tag="wtall")
            nc.scalar.dma_start(wtall[:, :TOTW // 2], dram["wtpack"][:, :TOTW // 2])
            nc.scalar.dma_start(wtall[:, TOTW // 2:], dram["wtpack"][:, TOTW // 2:])
            vecall = wt.tile([128, NVEC], F32, tag="vecall")
            nc.scalar.dma_start(vecall[:], dram["vecpack"][:])
            wdtall = wt.tile([R, 7 * DI], F16, tag="wdtall")
            nc.scalar.dma_start(wdtall[:], dram["wdtall"][:])
            wxTt, woutTt, wdtTt = [], [], []
            for i in range(7):
                o = i * BLKW
                wxTt.append(wtall[:, o:o + 128])
                woutTt.append(wtall[:, o + 128:o + BLKW])
                wdtTt.append(wdtall[:, i * DI:(i + 1) * DI])
            dcwTt, upwt, wgTt, dbTt = [], [], [], []
            for j in range(3):
                o = 7 * BLKW + j * GATW
                dcwTt.append(wtall[:, o:o + 384])
                upwt.append(wtall[:, o + 384:o + 640])
                wgTt.append(wtall[:, o + 640:o + 896])
                dbTt.append(wtall[:, o + 896:o + 1152])

            def vcol(i, g, c):
                o = i * 2 * NV + g * NV + c
                return vecall[:, o:o + 1]

            def gvcol(j, c):
                o = 7 * 2 * NV + j * 4 + c
                return vecall[:, o:o + 1]

            # per-block working tiles (double-buffered across blocks)
            def blk_tiles(i):
                xi = [blk.tile([128, L0 + 3], F16, tag=f"xi{g}",
                               name=f"xi{g}_{i}") for g in range(2)]
                z_t = [blk.tile([128, L0], F16, tag=f"z{g}", name=f"z{g}_{i}")
                       for g in range(2)]
                y_t = [blk.tile([128, L0], F16, tag=f"y{g}", name=f"y{g}_{i}")
                       for g in range(2)]
                xdbR = blk.tile([R, L0], F16, tag="xdbR", name=f"xdbR_{i}")
                bc16 = blk.tile([2 * NST, L0], F16, tag="bc16", name=f"bc16_{i}")
                u_t = [ubuf.tile([128, L0], F16, tag=f"u{g}", name=f"u{g}_{i}")
                       for g in range(2)]
                dt_t = [ubuf.tile([128, L0], F16, tag=f"dt{g}", name=f"dt{g}_{i}")
                        for g in range(2)]
                return xi, z_t, y_t, xdbR, bc16, u_t, dt_t

            carry = wt.tile([128, 2 * NST], F16, tag="carry")
            dA_g = [cube.tile([128, NST * TS], F16, tag=f"dA{g}",
                              name=f"dA{g}") for g in range(2)]
            dBu_g = [cube.tile([128, NST * TS], F16, tag=f"dBu{g}",
                               name=f"dBu{g}") for g in range(2)]
            repB = cube.tile([128, NST * TS], F16, tag="repB")
            repC = cube.tile([128, NST * TS], F16, tag="repC")

            def mamba(x_ap, i, Lb, out_ap, out_dma=None):
                cdw, winTb = preload.pop(i) if i in preload else load_blk(i)
                xi, z_t, y_t, xdbR, bc16, u_t, dt_t = blk_tiles(i)
                nc.vector.memset(xi[0][:, :3], 0.0)
                nc.vector.memset(xi[1][:, :3], 0.0)

                # ---- M1: in-proj (xi copies + z silus: one act table) ----
                for c0 in range(0, Lb, MM):
                    F = min(MM, Lb - c0)
                    for p in range(2):
                        ps = mmp.tile([128, MM], F32, tag="mmps")
                        nc.tensor.matmul(ps[:, :F], winTb[:, p * 128:(p + 1) * 128],
                                         x_ap[:, c0:c0 + F], start=True, stop=True)
                        nc.scalar.activation(xi[p][:, 3 + c0:3 + c0 + F],
                                             ps[:, :F], Act.Copy)
                    for g in range(2):
                        ps = mmp.tile([128, MM], F32, tag="mmps")
                        nc.tensor.matmul(ps[:, :F], winTb[:, (2 + g) * 128:(3 + g) * 128],
                                         x_ap[:, c0:c0 + F], start=True, stop=True)
                        nc.scalar.activation(z_t[g][:, c0:c0 + F], ps[:, :F],
                                             Act.Silu)
                # ---- M2: conv + u silu + x-proj (silu table) ----
                for c0 in range(0, Lb, MM):
                    F = min(MM, Lb - c0)
                    for g in range(2):
                        ps = mmp.tile([128, MM], F32, tag="mmps")
                        for k in range(KC):
                            nc.tensor.matmul(
                                ps[:, :F],
                                cdw[:, (g * KC + k) * 128:(g * KC + k + 1) * 128],
                                xi[g][:, c0 + k:c0 + k + F],
                                start=(k == 0), stop=(k == KC - 1))
                        nc.scalar.activation(u_t[g][:, c0:c0 + F], ps[:, :F],
                                             Act.Silu, bias=vcol(i, g, NST + 1))
                    psx = xdbp.tile([64, MM], F32, tag="xdbps")
                    for g in range(2):
                        nc.tensor.matmul(psx[:, :F],
                                         wxTt[i][:, g * 64:(g + 1) * 64],
                                         u_t[g][:, c0:c0 + F], start=(g == 0), stop=(g == 1))
                    nc.scalar.activation(xdbR[:, c0:c0 + F], psx[:R, :F], Act.Copy)
                    nc.scalar.activation(bc16[:, c0:c0 + F], psx[32:, :F], Act.Copy)
                    nc.sync.dma_start(bc_dram2[i % 2][:, c0:c0 + F], bc16[:, c0:c0 + F])
                # ---- M3: dt = softplus via exp+ln (nl_exp table) ----
                for c0 in range(0, Lb, MM):
                    F = min(MM, Lb - c0)
                    ztmp = cw.tile([128, MM], F16, tag="dtu", name="ztmpM")
                    for g in range(2):
                        ps = mmp.tile([128, MM], F32, tag="mmps")
                        nc.tensor.matmul(ps[:, :F], wdtTt[i][:, g * 128:(g + 1) * 128],
                                         xdbR[:, c0:c0 + F], start=True, stop=True)
                        nc.scalar.activation(ztmp[:, :F], ps[:, :F], Act.Exp,
                                             bias=vcol(i, g, NST + 2))
                        nc.scalar.activation(dt_t[g][:, c0:c0 + F], ztmp[:, :F],
                                             Act.Ln, bias=1.0)
                # ---- scan stage ----
                nchunks = (Lb + TS - 1) // TS
                for s in range(nchunks):
                    s0 = s * TS
                    F = min(TS, Lb - s0)
                    bc_dram = bc_dram2[i % 2]
                    bcz = bczp.tile([65, NST * TS], F16, tag="bcz")
                    nc.sync.dma_start(bcz[0:1, :NST * F], bc_dram[0:NST, s0:s0 + F])
                    nc.sync.dma_start(bcz[64:65, :NST * F], bc_dram[NST:, s0:s0 + F])
                    # broadcast B/C rows across partitions on the (idle)
                    # gpsimd engine; SBUF-resident bf16 keeps DVE muls in 2x
                    nc.gpsimd.partition_broadcast(repB[:, :NST * F],
                                                  bcz[0:1, :NST * F])
                    nc.gpsimd.partition_broadcast(repC[:, :NST * F],
                                                  bcz[64:65, :NST * F])
                    for g in range(2):
                        dA_t, dBu_t = dA_g[g], dBu_g[g]
                        dtu = cw.tile([128, TS], F16, tag="dtu")
                        nc.vector.tensor_mul(dtu[:, :F], dt_t[g][:, s0:s0 + F],
                                             u_t[g][:, s0:s0 + F])
                        for n in range(NST):
                            nc.scalar.activation(dA_t[:, n * F:(n + 1) * F],
                                                 dt_t[g][:, s0:s0 + F], Act.Exp,
                                                 scale=vcol(i, g, n))
                        nc.vector.tensor_mul(
                            dBu_t[:, :NST * F].rearrange("p (a b) -> p a b", a=NST),
                            dtu[:, :F].unsqueeze(1).broadcast_to([128, NST, F]),
                            repB[:, :NST * F].rearrange("p (a b) -> p a b", a=NST))
                        for n in range(NST):
                            init = 0.0 if s == 0 else carry[:, g * NST + n:g * NST + n + 1]
                            nc.vector.tensor_tensor_scan(
                                dBu_t[:, n * F:(n + 1) * F],
                                dA_t[:, n * F:(n + 1) * F],
                                dBu_t[:, n * F:(n + 1) * F],
                                init, op0=Alu.mult, op1=Alu.add)
                        if s + 1 < nchunks:
                            nc.vector.tensor_copy(carry[:, g * NST:(g + 1) * NST],
                                                  dBu_t[:, F - 1:NST * F:F])
                        nc.vector.tensor_mul(
                            dA_t[:, :NST * F].rearrange("p (a b) -> p a b", a=NST),
                            dBu_t[:, :NST * F].rearrange("p (a b) -> p a b", a=NST),
                            repC[:, :NST * F].rearrange("p (a b) -> p a b", a=NST))
                        nc.vector.tensor_add(dA_t[:, :8 * F], dA_t[:, :8 * F], dA_t[:, 8 * F:16 * F])
                        nc.vector.tensor_add(dA_t[:, :4 * F], dA_t[:, :4 * F], dA_t[:, 4 * F:8 * F])
                        nc.vector.tensor_add(dA_t[:, :2 * F], dA_t[:, :2 * F], dA_t[:, 2 * F:4 * F])
                        nc.vector.tensor_add(y_t[g][:, s0:s0 + F], dA_t[:, :F], dA_t[:, F:2 * F])
                # ---- O: y = (y + u*D) * silu(z); out-proj ----
                for c0 in range(0, Lb, MM):
                    F = min(MM, Lb - c0)
                    ps = mmp.tile([128, MM], F32, tag="mmps")
                    for g in range(2):
                        nc.vector.scalar_tensor_tensor(
                            y_t[g][:, c0:c0 + F], u_t[g][:, c0:c0 + F],
                            vcol(i, g, NST),
                            y_t[g][:, c0:c0 + F], op0=Alu.mult, op1=Alu.add)
                        nc.vector.tensor_mul(y_t[g][:, c0:c0 + F], y_t[g][:, c0:c0 + F],
                                             z_t[g][:, c0:c0 + F])
                        nc.tensor.matmul(ps[:, :F], woutTt[i][:, g * C:(g + 1) * C],
                                         y_t[g][:, c0:c0 + F], start=(g == 0), stop=(g == 1))
                    nc.scalar.activation(out_ap[:, c0:c0 + F], ps[:, :F], Act.Copy)
                    if out_dma is not None:
                        nc.sync.dma_start(out_dma[:, c0:c0 + F], out_ap[:, c0:c0 + F])

            def downconv(xt, off, j, Lb, out_ap):
                """xt: level tile; data at cols [off, off+Lb); front pad col off-1."""
                Lo = Lb // 2
                for c0 in range(0, Lo, MM):
                    F = min(MM, Lo - c0)
                    ps = mmp.tile([128, MM], F32, tag="mmps")
                    for k in range(3):
                        a = off + 2 * c0 + k - 1
                        nc.tensor.matmul(ps[:, :F], dcwTt[j][:, k * 128:(k + 1) * 128],
                                         xt[:, a:a + 2 * F - 1:2],
                                         start=(k == 0), stop=(k == 2))
                    nc.scalar.activation(out_ap[:, c0:c0 + F], ps[:, :F], Act.Identity,
                                         bias=gvcol(j, 0))

            def gate(t1_ap, t2_ap, j, Lb, f_ap):
                for c0 in range(0, Lb, MM):   # output chunk
                    F = min(MM, Lb - c0)
                    ch = c0 // 2
                    Fi = F // 2
                    t2u = gw.tile([128, MM], F16, tag="t2u")
                    pse = mmp.tile([128, MM], F32, tag="mmps")
                    nc.tensor.matmul(pse[:, :Fi], upwt[j][:, :128],
                                     t2_ap[:, ch:ch + Fi], start=True, stop=True)
                    nc.scalar.activation(t2u[:, 0:F:2], pse[:, :Fi], Act.Identity,
                                         bias=gvcol(j, 1))
                    pso = mmp.tile([128, MM], F32, tag="mmps")
                    nc.tensor.matmul(pso[:, :Fi], upwt[j][:, 128:],
                                     t2_ap[:, ch:ch + Fi], start=True, stop=True)
                    nc.scalar.activation(t2u[:, 1:F:2], pso[:, :Fi], Act.Identity,
                                         bias=gvcol(j, 1))
                    ps = mmp.tile([128, MM], F32, tag="mmps")
                    nc.tensor.matmul(ps[:, :F], wgTt[j][:, :128], t1_ap[:, c0:c0 + F],
                                     start=True, stop=False)
                    nc.tensor.matmul(ps[:, :F], wgTt[j][:, 128:], t2u[:, :F],
                                     start=False, stop=True)
                    wloc = gw.tile([128, MM], F16, tag="wloc")
                    nc.scalar.activation(wloc[:, :F], ps[:, :F], Act.Sigmoid,
                                         bias=gvcol(j, 2))
                    m1 = gw.tile([128, MM], F16, tag="m1")
                    m2 = gw.tile([128, MM], F16, tag="m2")
                    nc.vector.tensor_mul(m1[:, :F], t1_ap[:, c0:c0 + F], wloc[:, :F])
                    nc.vector.tensor_mul(m2[:, :F], t2u[:, :F], wloc[:, :F])
                    nc.vector.tensor_sub(m2[:, :F], t2u[:, :F], m2[:, :F])
                    ps2 = mmp.tile([128, MM], F32, tag="mmps")
                    nc.tensor.matmul(ps2[:, :F], dbTt[j][:, :128], m1[:, :F],
                                     start=True, stop=False)
                    nc.tensor.matmul(ps2[:, :F], dbTt[j][:, 128:], m2[:, :F],
                                     start=False, stop=True)
                    nc.scalar.activation(f_ap[:, c0:c0 + F], ps2[:, :F], Act.Identity,
                                         bias=gvcol(j, 3))

            # ---------- network ----------
            x1 = lvl.tile([128, 1025], F16, tag="x1")
            x2 = lvl.tile([128, 513], F16, tag="x2")
            x3 = lvl.tile([128, 257], F16, tag="x3")
            x4 = lvl.tile([128, 128], F16, tag="x4")
            e1 = lvl.tile([128, 1024], F16, tag="e1")
            e2 = lvl.tile([128, 512], F16, tag="e2")
            e3 = lvl.tile([128, 256], F16, tag="e3")
            e4 = lvl.tile([128, 128], F16, tag="e4")
            d4 = lvl.tile([128, 256], F16, tag="x3", name="d4")
            d3 = lvl.tile([128, 512], F16, tag="x2", name="d3")
            fbuf = lvl.tile([128, 1024], F16, tag="fbuf")
            outb = lvl.tile([128, 1024], F32, tag="outb")

            nc.vector.memset(x1[:, 0:1], 0.0)
            nc.vector.memset(x2[:, 0:1], 0.0)
            nc.vector.memset(x3[:, 0:1], 0.0)
            nc.sync.dma_start(x1[:, 1:1025], xT_d[:, :])

            mamba(x1[:, 1:1025], 0, 1024, e1[:, :])
            downconv(x1, 1, 0, 1024, x2[:, 1:513])
            mamba(x2[:, 1:513], 1, 512, e2[:, :])
            downconv(x2, 1, 1, 512, x3[:, 1:257])
            mamba(x3[:, 1:257], 2, 256, e3[:, :])
            downconv(x3, 1, 2, 256, x4[:, :])
            mamba(x4[:, :], 3, 128, e4[:, :])
            gate(e3[:, :], e4[:, :], 0, 256, fbuf[:, :256])
            mamba(fbuf[:, :256], 4, 256, d4[:, :])
            gate(e2[:, :], d4[:, :], 1, 512, fbuf[:, :512])
            mamba(fbuf[:, :512], 5, 512, d3[:, :])
            gate(e1[:, :], d3[:, :], 2, 1024, fbuf[:, :])
            mamba(fbuf[:, :], 6, 1024, outb[:, :], out_dma=out_d)

            # ---- dead-code microbench: gpsimd op rates for next iteration ----
            mb1 = mbp.tile([128, 2048], F16, tag="mb1")
            mb2 = mbp.tile([128, 2048], F16, tag="mb2")
            mb3 = mbp.tile([128, 1024], F32, tag="mb3")
            nc.vector.memset(mb1[:, :], 0.5)
            nc.vector.memset(mb3[:, :], 0.25)
            nc.gpsimd.partition_broadcast(mb2[:, :2048], mb1[0:1, :2048])
            nc.gpsimd.tensor_mul(mb1[:, :2048], mb1[:, :2048], mb2[:, :2048])
            nc.vector.tensor_tensor_scan(mb3[:, :512], mb3[:, :512],
                                         mb3[:, 512:1024], 0.0,
                                         op0=Alu.mult, op1=Alu.add)

    nc.compile()
    return nc


def _get_program():
    if "nc" not in _CACHE:
        _CACHE["nc"] = _build()
    return _CACHE["nc"]


def kernel(**inputs):
    from concourse.bass_utils import run_bass_kernel_spmd

    nc = _get_program()
    w = _prep_weights(inputs)
    x = np.asarray(inputs["x"], np.float32)  # [B, L, C]
    in_maps = []
    for c in range(NCORES):
        import ml_dtypes
        m = {"xT": np.ascontiguousarray(x[c % B].T).astype(ml_dtypes.bfloat16)}
        m.update(w)
        in_maps.append(m)
    res = run_bass_kernel_spmd(nc, in_maps, list(range(NCORES)))
    out = np.empty((B, L0, C), np.float32)
    for b in range(B):
        out[b] = res.results[b]["out"].T
    return out


# revision 13
# speedup vs baseline: 1.7118x; 1.7118x over previous
"""Trainium2 Bass kernel for the Mamba U-Net model (nn_Model_20770461843918).

SPMD over 8 NeuronCores: core c handles batch element c%4 and SSM-state
half r=c//4 (states [8r, 8r+8)).  The state split lives entirely in
per-core weight data (B/C rows of the x-projection, A-scale columns, D
masked to role 0), so the program is identical on all cores; one
AllReduce per mamba block over the (linear) output projection merges the
halves.  bf16 compute everywhere (fp32 PSUM accumulation, fp32 internal
scan state); the reference's activations decay to ~1e-13 so fp16 would
flush to zero — bf16 keeps fp32's exponent range.

  PE  : all matmuls bf16 (1 cyc/col); B/C broadcast via K=1 ones-matmul
  ACT : silu/exp/ln, PSUM->SBUF copies (act-table swaps minimized by
        splitting stage M into M1 in-proj / M2 conv+xproj / M3 dt passes)
  DVE : dtu, dBu = dtu*Brep (1 instr), 8 tensor_tensor_scan per group
        (bf16 operands, fp32 state), h*Crep (1 instr), tree-reduce
"""
import numpy as np

B, L0, C = 4, 1024, 128
DI, NST, R, KC = 256, 16, 8, 4
NSTL = 8              # states handled per core (16 split across core pairs)
NV = NSTL + 3         # per-partition vec cols: A[8], D, convb, bdt
NCORES = 8
TS = 512              # scan-stage time chunk
MM = 512              # matmul-stage time chunk

_CACHE = {}


def _prep_weights(inp, role):
    import ml_dtypes
    f32, f16 = np.float32, ml_dtypes.bfloat16
    g = lambda k: np.asarray(inp[k], f32)
    m_Win, m_convw, m_convb = g("m_Win"), g("m_convw"), g("m_convb")
    m_Wx, m_Wdt, m_bdt = g("m_Wx"), g("m_Wdt"), g("m_bdt")
    m_Alog, m_D, m_Wout = g("m_Alog"), g("m_D"), g("m_Wout")
    dc_w, dc_b = g("dc_w"), g("dc_b")
    wg_W, wg_b, db_W, db_b = g("wg_W"), g("wg_b"), g("db_W"), g("db_b")
    up_w, up_b = g("up_w"), g("up_b")

    winT = np.ascontiguousarray(m_Win.transpose(0, 2, 1)).astype(f16)  # [7,C,512]
    cd = np.zeros((7, 2, KC, 128, 128), f32)
    idx = np.arange(128)
    for i in range(7):
        for gg in range(2):
            for k in range(KC):
                cd[i, gg, k, idx, idx] = m_convw[i, gg * 128:(gg + 1) * 128, k]
    convdiag = np.ascontiguousarray(
        cd.transpose(0, 1, 3, 2, 4)).reshape(7, 2, 128, KC * 128).astype(f16)
    wxT_raw = np.ascontiguousarray(m_Wx.transpose(0, 2, 1)).reshape(7, 2, 128, R + 2 * NST)
    wxT = np.zeros((7, 2, 128, 64), f32)
    wxT[..., :R] = wxT_raw[..., :R]                       # dt -> psum parts 0..7
    s0 = NSTL * role
    wxT[..., 32:32 + NSTL] = wxT_raw[..., R + s0:R + s0 + NSTL]            # B local
    wxT[..., 40:40 + NSTL] = wxT_raw[..., R + NST + s0:R + NST + s0 + NSTL]  # C local
    wdtT = np.ascontiguousarray(m_Wdt.transpose(0, 2, 1))                # [7, R, DI]
    wdtall = wdtT.transpose(1, 0, 2).reshape(R, 7 * DI).astype(f16)     # [8, 7*256]
    woutT = np.ascontiguousarray(m_Wout.transpose(0, 2, 1)).reshape(7, 2, 128, C)
    dcwT = np.ascontiguousarray(dc_w.transpose(0, 2, 3, 1)).reshape(3, 128, 3 * 128)
    upw = np.ascontiguousarray(up_w.transpose(0, 1, 3, 2)).reshape(3, 128, 2 * 128)
    wgT = np.ascontiguousarray(wg_W.transpose(0, 2, 1)).reshape(3, 2, 128, 128)
    dbT = np.ascontiguousarray(db_W.transpose(0, 2, 1)).reshape(3, 2, 128, 128)

    panels = []
    for i in range(7):
        panels += [wxT[i, 0], wxT[i, 1], woutT[i, 0], woutT[i, 1]]
    for j in range(3):
        panels += [dcwT[j], upw[j], wgT[j, 0], wgT[j, 1], dbT[j, 0], dbT[j, 1]]
    wtpack = np.concatenate(panels, axis=1).astype(f16)

    # fp32 per-partition scalar columns (act scale/bias, stt scalars)
    A = -np.exp(m_Alog)                                                  # [7, DI, N]
    vec = np.zeros((7, 2, 128, NV), f32)
    for gg in range(2):
        sl = slice(gg * 128, (gg + 1) * 128)
        vec[:, gg, :, :NSTL] = A[:, sl, s0:s0 + NSTL]
        if role == 0:
            vec[:, gg, :, NSTL] = m_D[:, sl]             # u*D added by role 0 only
        vec[:, gg, :, NSTL + 1] = m_convb[:, sl]
        vec[:, gg, :, NSTL + 2] = m_bdt[:, sl]
    gv = np.zeros((3, 128, 4), f32)
    gv[:, :, 0], gv[:, :, 1], gv[:, :, 2], gv[:, :, 3] = dc_b, up_b, wg_b, db_b
    vecpack = np.concatenate(
        [vec.transpose(0, 1, 3, 2).reshape(7 * 2 * NV, 128).T,
         gv.transpose(0, 2, 1).reshape(12, 128).T], axis=1)

    return {"winT": np.ascontiguousarray(winT),
            "convdiag": np.ascontiguousarray(convdiag),
            "wdtall": np.ascontiguousarray(wdtall),
            "wtpack": np.ascontiguousarray(wtpack),
            "vecpack": np.ascontiguousarray(vecpack.astype(f32))}


def make_in_maps(inputs):
    import ml_dtypes
    x = np.asarray(inputs["x"], np.float32)  # [B, L, C]
    w_by_role = [_prep_weights(inputs, r) for r in range(2)]
    in_maps = []
    for c in range(NCORES):
        m = {"xT": np.ascontiguousarray(x[c % B].T).astype(ml_dtypes.bfloat16)}
        m.update(w_by_role[c // B])
        in_maps.append(m)
    return in_maps


def _build():
    import concourse.bacc as bacc
    import concourse.tile as tile
    import concourse.mybir as mybir

    F32 = mybir.dt.float32
    F16 = mybir.dt.bfloat16
    Alu = mybir.AluOpType
    Act = mybir.ActivationFunctionType
    GROUPS = [[c, c + B] for c in range(B)]   # state-half pairs

    nc = bacc.Bacc("TRN2", target_bir_lowering=False, debug=False,
                   num_devices=NCORES)

    xT_d = nc.declare_dram_parameter("xT", [C, L0], F16, isOutput=False)
    out_d = nc.declare_dram_parameter("out", [C, L0], F32, isOutput=True)
    BLKW, GATW = 384, 1152
    TOTW = 7 * BLKW + 3 * GATW
    NVEC = 7 * 2 * NV + 3 * 4
    dram = {}
    for name, shape, dt in [
        ("winT", [7, C, 2 * DI], F16), ("convdiag", [7, 2, 128, KC * 128], F16),
        ("wdtall", [R, 7 * DI], F16), ("wtpack", [128, TOTW], F16),
        ("vecpack", [128, NVEC], F32),
    ]:
        dram[name] = nc.declare_dram_parameter(name, shape, dt, isOutput=False)
    bc_dram2 = [nc.dram_tensor("bc_bounce0", [2 * NSTL, L0], F16),
                nc.dram_tensor("bc_bounce1", [2 * NSTL, L0], F16)]

    with tile.TileContext(nc) as tc:
        with tc.tile_pool(name="wt", bufs=1) as wt, \
             tc.tile_pool(name="lvl", bufs=1) as lvl, \
             tc.tile_pool(name="blk", bufs=2) as blk, \
             tc.tile_pool(name="cube", bufs=1) as cube, \
             tc.tile_pool(name="cw", bufs=2) as cw, \
             tc.tile_pool(name="ubuf", bufs=2) as ubuf, \
             tc.tile_pool(name="gw", bufs=2) as gw, \
             tc.tile_pool(name="cwc", bufs=2) as cwc, \
             tc.tile_pool(name="bczp", bufs=2) as bczp, \
             tc.tile_pool(name="drp", bufs=2, space="DRAM") as drp, \
             tc.tile_pool(name="mmp", bufs=3, space="PSUM") as mmp, \
             tc.tile_pool(name="xdbp", bufs=1, space="PSUM") as xdbp, \
             tc.tile_pool(name="repp", bufs=2, space="PSUM") as repp:

            ones2 = wt.tile([65, 128], F16, tag="ones2")
            nc.vector.memset(ones2[0:1, :], 1.0)
            nc.vector.memset(ones2[64:65, :], 1.0)

            def load_blk(i):
                winTb = cw.tile([C, 2 * DI], F16, tag="winT", name=f"winTb{i}")
                nc.scalar.dma_start(winTb[:], dram["winT"][i])
                cdw = cwc.tile([128, 2 * KC * 128], F16, tag="convdiag",
                               name=f"cdw{i}")
                nc.scalar.dma_start(cdw[:, :KC * 128], dram["convdiag"][i, 0])
                nc.scalar.dma_start(cdw[:, KC * 128:], dram["convdiag"][i, 1])
                return cdw, winTb

            preload = {0: load_blk(0)}

            wtall = wt.tile([128, TOTW], F16, tag="wtall")
            nc.scalar.dma_start(wtall[:, :TOTW // 2], dram["wtpack"][:, :TOTW // 2])
            nc.scalar.dma_start(wtall[:, TOTW // 2:], dram["wtpack"][:, TOTW // 2:])
            vecall = wt.tile([128, NVEC], F32, tag="vecall")
            nc.scalar.dma_start(vecall[:], dram["vecpack"][:])
            wdtall = wt.tile([R, 7 * DI], F16, tag="wdtall")
            nc.scalar.dma_start(wdtall[:], dram["wdtall"][:])
            wxTt, woutTt, wdtTt = [], [], []
            for i in range(7):
                o = i * BLKW
                wxTt.append(wtall[:, o:o + 128])
                woutTt.append(wtall[:, o + 128:o + BLKW])
                wdtTt.append(wdtall[:, i * DI:(i + 1) * DI])
            dcwTt, upwt, wgTt, dbTt = [], [], [], []
            for j in range(3):
                o = 7 * BLKW + j * GATW
                dcwTt.append(wtall[:, o:o + 384])
                upwt.append(wtall[:, o + 384:o + 640])
                wgTt.append(wtall[:, o + 640:o + 896])
                dbTt.append(wtall[:, o + 896:o + 1152])

            def vcol(i, g, c):
                o = i * 2 * NV + g * NV + c
                return vecall[:, o:o + 1]

            def gvcol(j, c):
                o = 7 * 2 * NV + j * 4 + c
                return vecall[:, o:o + 1]

            def blk_tiles(i):
                xi = [blk.tile([128, L0 + 3], F16, tag=f"xi{g}",
                               name=f"xi{g}_{i}") for g in range(2)]
                z_t = [blk.tile([128, L0], F16, tag=f"z{g}", name=f"z{g}_{i}")
                       for g in range(2)]
                y_t = [blk.tile([128, L0], F16, tag=f"y{g}", name=f"y{g}_{i}")
                       for g in range(2)]
                xdbR = blk.tile([R, L0], F16, tag="xdbR", name=f"xdbR_{i}")
                bc16 = blk.tile([2 * NSTL, L0], F16, tag="bc16", name=f"bc16_{i}")
                u_t = [ubuf.tile([128, L0], F16, tag=f"u{g}", name=f"u{g}_{i}")
                       for g in range(2)]
                dt_t = [ubuf.tile([128, L0], F16, tag=f"dt{g}", name=f"dt{g}_{i}")
                        for g in range(2)]
                return xi, z_t, y_t, xdbR, bc16, u_t, dt_t

            carry = wt.tile([128, 2 * NSTL], F16, tag="carry")
            dA_g = [cube.tile([128, NSTL * TS], F16, tag=f"dA{g}",
                              name=f"dA{g}") for g in range(2)]
            dBu_g = [cube.tile([128, NSTL * TS], F16, tag=f"dBu{g}",
                               name=f"dBu{g}") for g in range(2)]
            repB = cube.tile([128, NSTL * TS], F16, tag="repB")
            repC = cube.tile([128, NSTL * TS], F16, tag="repC")

            def mamba(x_ap, i, Lb, out_ap, final=False):
                cdw, winTb = preload.pop(i) if i in preload else load_blk(i)
                xi, z_t, y_t, xdbR, bc16, u_t, dt_t = blk_tiles(i)
                nc.vector.memset(xi[0][:, :3], 0.0)
                nc.vector.memset(xi[1][:, :3], 0.0)

                # ---- M1: in-proj (xi copies + z silus) ----
                for c0 in range(0, Lb, MM):
                    F = min(MM, Lb - c0)
                    for p in range(2):
                        ps = mmp.tile([128, MM], F32, tag="mmps")
                        nc.tensor.matmul(ps[:, :F], winTb[:, p * 128:(p + 1) * 128],
                                         x_ap[:, c0:c0 + F], start=True, stop=True)
                        nc.scalar.activation(xi[p][:, 3 + c0:3 + c0 + F],
                                             ps[:, :F], Act.Copy)
                    for g in range(2):
                        ps = mmp.tile([128, MM], F32, tag="mmps")
                        nc.tensor.matmul(ps[:, :F], winTb[:, (2 + g) * 128:(3 + g) * 128],
                                         x_ap[:, c0:c0 + F], start=True, stop=True)
                        nc.scalar.activation(z_t[g][:, c0:c0 + F], ps[:, :F],
                                             Act.Silu)
                # ---- M2: conv + u silu + x-proj ----
                for c0 in range(0, Lb, MM):
                    F = min(MM, Lb - c0)
                    for g in range(2):
                        ps = mmp.tile([128, MM], F32, tag="mmps")
                        for k in range(KC):
                            nc.tensor.matmul(
                                ps[:, :F],
                                cdw[:, (g * KC + k) * 128:(g * KC + k + 1) * 128],
                                xi[g][:, c0 + k:c0 + k + F],
                                start=(k == 0), stop=(k == KC - 1))
                        nc.scalar.activation(u_t[g][:, c0:c0 + F], ps[:, :F],
                                             Act.Silu, bias=vcol(i, g, NSTL + 1))
                    psx = xdbp.tile([64, MM], F32, tag="xdbps")
                    for g in range(2):
                        nc.tensor.matmul(psx[:, :F],
                                         wxTt[i][:, g * 64:(g + 1) * 64],
                                         u_t[g][:, c0:c0 + F], start=(g == 0), stop=(g == 1))
                    nc.scalar.activation(xdbR[:, c0:c0 + F], psx[:R, :F], Act.Copy)
                    nc.scalar.activation(bc16[:, c0:c0 + F], psx[32:48, :F], Act.Copy)
                    nc.sync.dma_start(bc_dram2[i % 2][:, c0:c0 + F], bc16[:, c0:c0 + F])
                # ---- M3: dt = softplus via exp+ln ----
                for c0 in range(0, Lb, MM):
                    F = min(MM, Lb - c0)
                    ztmp = cw.tile([128, MM], F16, tag="dtu", name="ztmpM")
                    for g in range(2):
                        ps = mmp.tile([128, MM], F32, tag="mmps")
                        nc.tensor.matmul(ps[:, :F], wdtTt[i][:, g * 128:(g + 1) * 128],
                                         xdbR[:, c0:c0 + F], start=True, stop=True)
                        nc.scalar.activation(ztmp[:, :F], ps[:, :F], Act.Exp,
                                             bias=vcol(i, g, NSTL + 2))
                        nc.scalar.activation(dt_t[g][:, c0:c0 + F], ztmp[:, :F],
                                             Act.Ln, bias=1.0)
                # ---- scan stage (local 8 states) ----
                nchunks = (Lb + TS - 1) // TS
                for s in range(nchunks):
                    s0 = s * TS
                    F = min(TS, Lb - s0)
                    bc_dram = bc_dram2[i % 2]
                    bcz = bczp.tile([65, NSTL * TS], F16, tag="bcz")
                    nc.sync.dma_start(bcz[0:1, :NSTL * F], bc_dram[0:NSTL, s0:s0 + F])
                    nc.sync.dma_start(bcz[64:65, :NSTL * F], bc_dram[NSTL:, s0:s0 + F])
                    # broadcast local B/C rows across partitions (PE + ACT copy)
                    for row, dst in ((0, repB), (64, repC)):
                        for n0 in range(0, NSTL, 2):
                            rp = repp.tile([128, 2 * TS], F32, tag="rep")
                            nc.tensor.matmul(rp[:, :F], ones2[row:row + 1, :],
                                             bcz[row:row + 1, n0 * F:(n0 + 1) * F],
                                             start=True, stop=True)
                            nc.tensor.matmul(rp[:, F:2 * F], ones2[row:row + 1, :],
                                             bcz[row:row + 1, (n0 + 1) * F:(n0 + 2) * F],
                                             start=True, stop=True)
                            nc.scalar.activation(dst[:, n0 * F:(n0 + 2) * F],
                                                 rp[:, :2 * F], Act.Copy)
                    for g in range(2):
                        dA_t, dBu_t = dA_g[g], dBu_g[g]
                        dtu = cw.tile([128, TS], F16, tag="dtu")
                        nc.vector.tensor_mul(dtu[:, :F], dt_t[g][:, s0:s0 + F],
                                             u_t[g][:, s0:s0 + F])
                        for n in range(NSTL):
                            nc.scalar.activation(dA_t[:, n * F:(n + 1) * F],
                                                 dt_t[g][:, s0:s0 + F], Act.Exp,
                                                 scale=vcol(i, g, n))
                        nc.vector.tensor_mul(
                            dBu_t[:, :NSTL * F].rearrange("p (a b) -> p a b", a=NSTL),
                            dtu[:, :F].unsqueeze(1).broadcast_to([128, NSTL, F]),
                            repB[:, :NSTL * F].rearrange("p (a b) -> p a b", a=NSTL))
                        for n in range(NSTL):
                            init = 0.0 if s == 0 else carry[:, g * NSTL + n:g * NSTL + n + 1]
                            nc.vector.tensor_tensor_scan(
                                dBu_t[:, n * F:(n + 1) * F],
                                dA_t[:, n * F:(n + 1) * F],
                                dBu_t[:, n * F:(n + 1) * F],
                                init, op0=Alu.mult, op1=Alu.add)
                        if s + 1 < nchunks:
                            nc.vector.tensor_copy(carry[:, g * NSTL:(g + 1) * NSTL],
                                                  dBu_t[:, F - 1:NSTL * F:F])
                        nc.vector.tensor_mul(
                            dA_t[:, :NSTL * F].rearrange("p (a b) -> p a b", a=NSTL),
                            dBu_t[:, :NSTL * F].rearrange("p (a b) -> p a b", a=NSTL),
                            repC[:, :NSTL * F].rearrange("p (a b) -> p a b", a=NSTL))
                        nc.vector.tensor_add(dA_t[:, :4 * F], dA_t[:, :4 * F], dA_t[:, 4 * F:8 * F])
                        nc.vector.tensor_add(dA_t[:, :2 * F], dA_t[:, :2 * F], dA_t[:, 2 * F:4 * F])
                        nc.vector.tensor_add(y_t[g][:, s0:s0 + F], dA_t[:, :F], dA_t[:, F:2 * F])
                # ---- O: y = (y_local + u*D_role) * silu(z); out-proj; AllReduce ----
                ydt = F32 if final else F16
                yout = blk.tile([128, L0], ydt, tag="yout32" if final else "yout",
                                name=f"yout_{i}")
                for c0 in range(0, Lb, MM):
                    F = min(MM, Lb - c0)
                    ps = mmp.tile([128, MM], F32, tag="mmps")
                    for g in range(2):
                        nc.vector.scalar_tensor_tensor(
                            y_t[g][:, c0:c0 + F], u_t[g][:, c0:c0 + F],
                            vcol(i, g, NSTL),
                            y_t[g][:, c0:c0 + F], op0=Alu.mult, op1=Alu.add)
                        nc.vector.tensor_mul(y_t[g][:, c0:c0 + F], y_t[g][:, c0:c0 + F],
                                             z_t[g][:, c0:c0 + F])
                        nc.tensor.matmul(ps[:, :F], woutTt[i][:, g * C:(g + 1) * C],
                                         y_t[g][:, c0:c0 + F], start=(g == 0), stop=(g == 1))
                    nc.scalar.activation(yout[:, c0:c0 + F], ps[:, :F], Act.Copy)
                # merge state-halves across the core pair
                suf = f"{Lb}f" if final else f"{Lb}"
                arin = drp.tile([128, Lb], ydt, tag=f"arin{suf}",
                                name=f"arin_{i}")
                arout = drp.tile([128, Lb], ydt, tag=f"arout{suf}",
                                 name=f"arout_{i}")
                nc.gpsimd.dma_start(arin[:, :Lb], yout[:, :Lb])
                nc.gpsimd.collective_compute(
                    "AllReduce", Alu.add, replica_groups=GROUPS,
                    ins=[arin[:, :Lb].opt()], outs=[arout[:, :Lb].opt()])
                if final:
                    nc.gpsimd.dma_start(out_ap[:, :Lb], arout[:, :Lb])
                    nc.sync.dma_start(out_d[:, :], out_ap[:, :Lb])
                else:
                    nc.gpsimd.dma_start(out_ap[:, :Lb], arout[:, :Lb])

            def downconv(xt, off, j, Lb, out_ap):
                """xt: level tile; data at cols [off, off+Lb); front pad col off-1."""
                Lo = Lb // 2
                for c0 in range(0, Lo, MM):
                    F = min(MM, Lo - c0)
                    ps = mmp.tile([128, MM], F32, tag="mmps")
                    for k in range(3):
                        a = off + 2 * c0 + k - 1
                        nc.tensor.matmul(ps[:, :F], dcwTt[j][:, k * 128:(k + 1) * 128],
                                         xt[:, a:a + 2 * F - 1:2],
                                         start=(k == 0), stop=(k == 2))
                    nc.scalar.activation(out_ap[:, c0:c0 + F], ps[:, :F], Act.Identity,
                                         bias=gvcol(j, 0))

            def gate(t1_ap, t2_ap, j, Lb, f_ap):
                for c0 in range(0, Lb, MM):   # output chunk
                    F = min(MM, Lb - c0)
                    ch = c0 // 2
                    Fi = F // 2
                    t2u = gw.tile([128, MM], F16, tag="t2u")
                    pse = mmp.tile([128, MM], F32, tag="mmps")
                    nc.tensor.matmul(pse[:, :Fi], upwt[j][:, :128],
                                     t2_ap[:, ch:ch + Fi], start=True, stop=True)
                    nc.scalar.activation(t2u[:, 0:F:2], pse[:, :Fi], Act.Identity,
                                         bias=gvcol(j, 1))
                    pso = mmp.tile([128, MM], F32, tag="mmps")
                    nc.tensor.matmul(pso[:, :Fi], upwt[j][:, 128:],
                                     t2_ap[:, ch:ch + Fi], start=True, stop=True)
                    nc.scalar.activation(t2u[:, 1:F:2], pso[:, :Fi], Act.Identity,
                                         bias=gvcol(j, 1))
                    ps = mmp.tile([128, MM], F32, tag="mmps")
                    nc.tensor.matmul(ps[:, :F], wgTt[j][:, :128], t1_ap[:, c0:c0 + F],
                                     start=True, stop=False)
                    nc.tensor.matmul(ps[:, :F], wgTt[j][:, 128:], t2u[:, :F],
                                     start=False, stop=True)
                    wloc = gw.tile([128, MM], F16, tag="wloc")
                    nc.scalar.activation(wloc[:, :F], ps[:, :F], Act.Sigmoid,
                                         bias=gvcol(j, 2))
                    m1 = gw.tile([128, MM], F16, tag="m1")
                    m2 = gw.tile([128, MM], F16, tag="m2")
                    nc.vector.tensor_mul(m1[:, :F], t1_ap[:, c0:c0 + F], wloc[:, :F])
                    nc.vector.tensor_mul(m2[:, :F], t2u[:, :F], wloc[:, :F])
                    nc.vector.tensor_sub(m2[:, :F], t2u[:, :F], m2[:, :F])
                    ps2 = mmp.tile([128, MM], F32, tag="mmps")
                    nc.tensor.matmul(ps2[:, :F], dbTt[j][:, :128], m1[:, :F],
                                     start=True, stop=False)
                    nc.tensor.matmul(ps2[:, :F], dbTt[j][:, 128:], m2[:, :F],
                                     start=False, stop=True)
                    nc.scalar.activation(f_ap[:, c0:c0 + F], ps2[:, :F], Act.Identity,
                                         bias=gvcol(j, 3))

            # ---------- network ----------
            x1 = lvl.tile([128, 1025], F16, tag="x1")
            x2 = lvl.tile([128, 513], F16, tag="x2")
            x3 = lvl.tile([128, 257], F16, tag="x3")
            x4 = lvl.tile([128, 128], F16, tag="x4")
            e1 = lvl.tile([128, 1024], F16, tag="e1")
            e2 = lvl.tile([128, 512], F16, tag="e2")
            e3 = lvl.tile([128, 256], F16, tag="e3")
            e4 = lvl.tile([128, 128], F16, tag="e4")
            d4 = lvl.tile([128, 256], F16, tag="x3", name="d4")
            d3 = lvl.tile([128, 512], F16, tag="x2", name="d3")
            fbuf = lvl.tile([128, 1024], F16, tag="fbuf")
            outb = lvl.tile([128, 1024], F32, tag="outb")

            nc.vector.memset(x1[:, 0:1], 0.0)
            nc.vector.memset(x2[:, 0:1], 0.0)
            nc.vector.memset(x3[:, 0:1], 0.0)
            nc.sync.dma_start(x1[:, 1:1025], xT_d[:, :])

            mamba(x1[:, 1:1025], 0, 1024, e1[:, :])
            downconv(x1, 1, 0, 1024, x2[:, 1:513])
            mamba(x2[:, 1:513], 1, 512, e2[:, :])
            downconv(x2, 1, 1, 512, x3[:, 1:257])
            mamba(x3[:, 1:257], 2, 256, e3[:, :])
            downconv(x3, 1, 2, 256, x4[:, :])
            mamba(x4[:, :], 3, 128, e4[:, :])
            gate(e3[:, :], e4[:, :], 0, 256, fbuf[:, :256])
            mamba(fbuf[:, :256], 4, 256, d4[:, :])
            gate(e2[:, :], d4[:, :], 1, 512, fbuf[:, :512])
            mamba(fbuf[:, :512], 5, 512, d3[:, :])
            gate(e1[:, :], d3[:, :], 2, 1024, fbuf[:, :])
            mamba(fbuf[:, :], 6, 1024, outb[:, :], final=True)

    nc.compile()
    return nc


def _get_program():
    if "nc" not in _CACHE:
        _CACHE["nc"] = _build()
    return _CACHE["nc"]


def kernel(**inputs):
    from concourse.bass_utils import run_bass_kernel_spmd

    nc = _get_program()
    in_maps = make_in_maps(inputs)
    res = run_bass_kernel_spmd(nc, in_maps, list(range(NCORES)))
    out = np.empty((B, L0, C), np.float32)
    for b in range(B):
        out[b] = res.results[b]["out"].T
    return out
